# revision 41
# baseline (speedup 1.0000x reference)
"""Causal multi-head attention block (QKV proj + causal softmax attention + out proj)
for Trainium2, sharded over 8 NeuronCores: data-parallel over batch (2), tensor-
parallel over heads (16 heads -> 4 per core).

Shapes (hardcoded): B=2, T=2048, C=1024, H=16, Dh=64.
Each core computes a partial output projection [T, C] for its 4 heads; the host
sums the 4 partials per batch and adds the fc bias.

Schedule: per t-chunk, the next chunk's QKV projection and the previous chunk's
FC are interleaved as PE filler units inside the attention pair stream, so the
tensor engine never waits on the softmax (exp on scalar, mask/normalize on
vector) dependency chain. Softmax denominators are reciprocal'd in a
transposed [128,4] layout (DRAM bounce on the sync HW DGE queue) mid-kernel;
the final head-pair instead uses an engine-only 32x32 stream-transpose chain
(the AV matmul replicates the denominator into 32 psum rows), emitted per
256-column half as soon as its last AV lands, so the kernel tail has no
DMA-semaphore latency in its critical path and the last FC units start as
early as the slice dependencies allow.
"""

import os

import numpy as np

import concourse.bass as bass
import concourse.tile as tile
from concourse import bacc, mybir
from concourse.bass_utils import run_bass_kernel_spmd

F32 = mybir.dt.float32
BF16 = mybir.dt.bfloat16

B = 2
T = 2048
C = 1024
H_PER_CORE = 4  # local heads per core
DH = 64
O_CORE = H_PER_CORE * DH  # 256 output channels per core (per q/k/v)

TCH = 512  # t-chunk size (free dim of most matmuls)
N_CHUNKS = T // TCH  # 4
KT = T // 128  # 16 k-tiles of 128

_BUILD_CACHE = {}
LAST_RESULT = None


def build(t=T):
    n_chunks = t // TCH
    nc = bacc.Bacc("TRN2", target_bir_lowering=False)

    xtl = nc.declare_dram_parameter("xtl", [t // TCH, 128, 8, TCH], BF16, isOutput=False)
    wqkl = nc.declare_dram_parameter("wqkl", [128, 8, 2 * O_CORE], BF16, isOutput=False)
    wvl = nc.declare_dram_parameter("wvl", [128, 8, O_CORE], BF16, isOutput=False)
    bqk = nc.declare_dram_parameter("bqk", [128, 4], F32, isOutput=False)
    bv_rep = nc.declare_dram_parameter("bv_rep", [128, O_CORE], F32, isOutput=False)
    wfcl = nc.declare_dram_parameter("wfcl", [128, 2, C], BF16, isOutput=False)
    mask = nc.declare_dram_parameter("mask", [128, 128], BF16, isOutput=False)
    y = nc.declare_dram_parameter("y", [t, C], BF16, isOutput=True)

    with (
        tile.TileContext(nc) as tc,
        tc.tile_pool(name="singles", bufs=1) as singles,
        tc.tile_pool(name="xpool", bufs=3) as xpool,
        tc.tile_pool(name="wtpool", bufs=8) as wtpool,
        tc.tile_pool(name="attnpool", bufs=3) as attnpool,
        tc.tile_pool(name="opool", bufs=4) as opool,
        tc.tile_pool(name="rpool", bufs=8) as rpool,
        tc.tile_pool(name="dpool", bufs=32, space="DRAM") as dpool,
        tc.tile_pool(name="mmps", bufs=2, space="PSUM") as mmps,
        tc.tile_pool(name="sps", bufs=2, space="PSUM") as sps,
        tc.tile_pool(name="avps", bufs=2, space="PSUM") as avps,
    ):
        # ---- persistent SBUF tensors ----
        wqk_sb = singles.tile([128, 8, 2 * O_CORE], BF16)  # [ci, co, qk]
        wv_sb = singles.tile([128, 8, O_CORE], BF16)  # [ci, co, v]
        xt0 = xpool.tile([128, 8, TCH], BF16, tag="xt", name="xt0")
        bqk_sb = singles.tile([128, 4], F32)
        bv_sb = singles.tile([128, H_PER_CORE, DH], F32)
        mask_sb = singles.tile([128, 128], BF16)
        wfc_sb = singles.tile([128, 2, C], BF16)  # [p, ks, n]

        # Startup loads: host-pretiled (full-line-rate DMAs). The scalar HW
        # queue carries ONLY the wq loads and the sync queue leads with xt0,
        # so the first QKV matmuls' coalesced queue-counter waits cover
        # nothing but their own inputs.
        nc.scalar.dma_start(wqk_sb[:, 0:4, :], wqkl[:, 0:4, :])
        nc.scalar.dma_start(xt0[:, 4:8, :], xtl[0, :, 4:8, :])
        nc.scalar.dma_start(wqk_sb[:, 4:8, :], wqkl[:, 4:8, :])
        nc.sync.dma_start(xt0[:, 0:4, :], xtl[0, :, 0:4, :])
        nc.sync.dma_start(wv_sb[:], wvl[:])
        nc.sync.dma_start(bqk_sb[:], bqk[:])
        nc.sync.dma_start(mask_sb[:], mask[:])
        nc.sync.dma_start(bv_sb[:], bv_rep.rearrange("p (h d) -> p h d", h=H_PER_CORE))
        nc.sync.dma_start(wfc_sb[:], wfcl[:])

        qT_sb = singles.tile([128, 2, t], BF16)  # [dh + 64*(h%2), h//2, t]
        kT_sb = singles.tile([128, 2, t], BF16)
        v_sb = singles.tile([128, t // 128, H_PER_CORE, DH + 32], BF16)  # [k_in, kt, h, d|1s]
        nc.vector.memset(v_sb[:, :, :, DH : DH + 32], 1.0)  # 32 ones cols -> denom x32

        def qk_dst(i):
            return (qT_sb if i < 2 else kT_sb), i % 2

        # ---- chunk 0 QKV: co-major so matmuls start as soon as the first
        # co-slices of x and w land; 4 QK chains + 2 V chains accumulate
        # simultaneously across 6 psum banks.
        ps_qk2 = [
            sps.tile([128, 2, TCH], F32, tag="s", name="qks0"),
            sps.tile([128, 2, TCH], F32, tag="s", name="qks1"),
        ]
        # each interleaved accumulation chain must own a full PSUM bank:
        # interleaving two start/stop groups within one bank corrupts the
        # earlier group's start (observed on HW)
        ps_v = [
            mmps.tile([128, TCH], F32, tag="mm", name="vps0"),
            mmps.tile([128, TCH], F32, tag="mm", name="vps1"),
            avps.tile([128, TCH], F32, tag="av", name="vps2"),
            avps.tile([128, TCH], F32, tag="av", name="vps3"),
        ]
        for co in range(8):
            for i in range(4):
                nc.tensor.matmul(
                    ps_qk2[i // 2][:, i % 2, :],
                    wqk_sb[:, co, i * 128 : (i + 1) * 128],
                    xt0[:, co, :],
                    start=(co == 0),
                    stop=(co == 7),
                    skip_group_check=True,
                )
        for co in range(8):
            for tt in range(4):
                nc.tensor.matmul(
                    ps_v[tt][:, 0:256],
                    xt0[:, co, tt * 128 : (tt + 1) * 128],
                    wv_sb[:, co, :],
                    start=(co == 0),
                    stop=(co == 7),
                    skip_group_check=True,
                )
        for i in range(4):
            dst, half = qk_dst(i)
            nc.vector.tensor_scalar_add(
                dst[:, half, 0:TCH], ps_qk2[i // 2][:, i % 2, :], bqk_sb[:, i : i + 1]
            )
        for tt in range(4):
            nc.vector.tensor_add(
                v_sb[:, tt, :, 0:DH],
                ps_v[tt][:, 0:256].rearrange("p (h d) -> p h d", h=H_PER_CORE),
                bv_sb[:],
            )

        def load_xt(tcix):
            xt = xpool.tile([128, 8, TCH], BF16, tag="xt", name=f"xt{tcix}")
            nc.gpsimd.dma_start(xt[:, 0:4, :], xtl[tcix, :, 0:4, :])
            nc.gpsimd.dma_start(xt[:, 4:8, :], xtl[tcix, :, 4:8, :])
            return xt

        def qkv_units(tcix, xt):
            """Filler units (each a self-contained psum chain) for chunk tcix's
            QKV projection, interleaved into the previous chunk's attention."""
            ts0 = tcix * TCH

            def qk_group(i):
                ps = mmps.tile([128, TCH], F32, tag="mm", name="qkps")
                for co in range(8):
                    nc.tensor.matmul(
                        ps[:],
                        wqk_sb[:, co, i * 128 : (i + 1) * 128],
                        xt[:, co, :],
                        start=(co == 0),
                        stop=(co == 7),
                        skip_group_check=True,
                    )
                dst, half = qk_dst(i)
                nc.vector.tensor_scalar_add(
                    dst[:, half, ts0 : ts0 + TCH], ps[:], bqk_sb[:, i : i + 1]
                )

            def v_group(j):
                ps = mmps.tile([128, TCH], F32, tag="mm", name="vps")
                for half in range(2):
                    tt = j * 2 + half
                    for co in range(8):
                        nc.tensor.matmul(
                            ps[:, half * 256 : (half + 1) * 256],
                            xt[:, co, tt * 128 : (tt + 1) * 128],
                            wv_sb[:, co, :],
                            start=(co == 0),
                            stop=(co == 7),
                            skip_group_check=True,
                        )
                kt0 = tcix * 4 + j * 2
                nc.vector.tensor_add(
                    v_sb[:, kt0 : kt0 + 2, :, 0:DH],
                    ps.rearrange("p (a h d) -> p a h d", a=2, h=H_PER_CORE),
                    bv_sb[:, None, :, :].to_broadcast((128, 2, H_PER_CORE, DH)),
                )

            units = [lambda i=i: qk_group(i) for i in range(4)]
            units += [lambda j=j: v_group(j) for j in range(2)]
            return units

        attn_tiles = {}

        def fc_units(tcix):
            """Filler units for chunk tcix's output projection."""
            ts0 = tcix * TCH
            at = attn_tiles[tcix]
            big = tcix == n_chunks - 1
            units = []
            for tt in range(4):
                for nn in range(2):
                    def u(tt=tt, nn=nn):
                        pool = avps if (big and (tt + nn) % 2 == 1) else mmps
                        tg = "av" if (big and (tt + nn) % 2 == 1) else "mm"
                        ps = pool.tile([128, TCH], F32, tag=tg, name="fcps")
                        for ks in range(2):
                            nc.tensor.matmul(
                                ps[:],
                                at[:, ks, tt * 128 : (tt + 1) * 128],
                                wfc_sb[:, ks, nn * TCH : (nn + 1) * TCH],
                                start=(ks == 0),
                                stop=(ks == 1),
                                skip_group_check=True,
                            )
                        ot = opool.tile([128, TCH], BF16, tag="o")
                        nc.vector.tensor_copy(ot[:], ps[:])
                        last = tcix == n_chunks - 1
                        eng = nc.scalar if last else nc.gpsimd
                        eng.dma_start(
                            y[ts0 + tt * 128 : ts0 + (tt + 1) * 128,
                              nn * TCH : (nn + 1) * TCH],
                            ot[:],
                        )
                    units.append(u)
            return units

        # ---- main loop: attention per chunk with filler interleaving ----
        for tcix in range(n_chunks):
            ts0 = tcix * TCH
            fillers = []
            if tcix + 1 < n_chunks:
                fillers += qkv_units(tcix + 1, load_xt(tcix + 1))
            if tcix >= 1:
                fillers += fc_units(tcix - 1)

            kimax = tcix * 4 + 3
            npairs = 2 * tcix + 2
            slots = 2 * npairs * 2
            state = {"s": 0, "f": 0}

            def tick():
                state["s"] += 1
                want = state["s"] * len(fillers) // slots
                while state["f"] < want:
                    fillers[state["f"]]()
                    state["f"] += 1

            attn_t = attnpool.tile([128, 2, TCH], BF16, tag="attn")
            attn_tiles[tcix] = attn_t
            for hp in range(2):
                heads = (2 * hp, 2 * hp + 1)
                ps_av = {}
                for h in heads:
                    ps_av[h] = avps.tile([128, TCH], F32, tag="av", name=f"av{h}")
                wts = {}
                pend = []

                tail = tcix == n_chunks - 1 and hp == 1
                mv = DH + 32 if tail else DH + 1

                def do_av(item):
                    h, pj = item
                    wt = wts[(h, pj)]
                    for u in range(2):
                        ki = 2 * pj + u
                        sx = max(0, (ki - 4 * tcix) * 128)
                        nc.tensor.matmul(
                            ps_av[h][0:mv, sx:TCH],
                            v_sb[:, ki, h, 0:mv],
                            wt[:, u, sx:TCH],
                            start=(ki == 0),
                            stop=(ki == kimax),
                            skip_group_check=True,
                        )

                def emit_half(h, c0):
                    pb = (h % 2) * 64
                    ho = h // 2
                    t1 = rpool.tile([32, 256], F32, tag="t1")
                    nc.vector.transpose(t1[:], ps_av[h][DH : DH + 32, c0 : c0 + 256])
                    t1v = t1.rearrange("p (j k) -> p j k", k=32)
                    rcp32 = rpool.tile([32, 8], F32, tag="rcp32")
                    nc.vector.reciprocal(rcp32[:], t1v[:, :, 0:1])
                    in2 = rpool.tile([32, 256], F32, tag="in2")
                    nc.vector.tensor_copy(
                        in2.rearrange("p (j k) -> p j k", k=32),
                        rcp32[:, :, None].to_broadcast((32, 8, 32)),
                    )
                    out2 = rpool.tile([32, 256], F32, tag="out2")
                    nc.vector.transpose(out2[:], in2[:])
                    nc.vector.tensor_mul(
                        attn_t[pb : pb + 32, ho, c0 : c0 + 256],
                        ps_av[h][0:32, c0 : c0 + 256],
                        out2[:],
                    )
                    nc.vector.tensor_mul(
                        attn_t[pb + 32 : pb + 64, ho, c0 : c0 + 256],
                        ps_av[h][32:DH, c0 : c0 + 256],
                        out2[:],
                    )

                def finish(item):
                    do_av(item)
                    if tail:
                        hh, pjj = item
                        if pjj == npairs - 2:
                            emit_half(hh, 0)
                        elif pjj == npairs - 1:
                            emit_half(hh, 256)

                for pj in range(npairs):
                    for h in heads:
                        pb = (h % 2) * 64
                        ho = h // 2
                        st = sps.tile([128, 2, TCH], F32, tag="s")
                        for u in range(2):
                            ki = 2 * pj + u
                            # columns left of the diagonal block are fully
                            # masked and never read downstream: skip them
                            m = ki - 4 * tcix
                            sx = 128 * m if m >= 1 else 0
                            nc.tensor.matmul(
                                st[:, u, sx:],
                                kT_sb[pb : pb + 64, ho, ki * 128 : (ki + 1) * 128],
                                qT_sb[pb : pb + 64, ho, ts0 + sx : ts0 + TCH],
                                start=True,
                                stop=True,
                                skip_group_check=True,
                            )
                        wt = wtpool.tile([128, 2, TCH], BF16, tag="wt")
                        m0 = 2 * pj - 4 * tcix  # block offset of the pair's first ki
                        if m0 >= 2:
                            # mostly-masked final pair: exp only the live columns
                            for u in range(2):
                                sx = (m0 + u) * 128
                                nc.scalar.activation(
                                    wt[:, u, sx:TCH],
                                    st[:, u, sx:TCH],
                                    mybir.ActivationFunctionType.Exp,
                                    scale=0.125,
                                )
                        else:
                            nc.scalar.activation(
                                wt[:],
                                st[:],
                                mybir.ActivationFunctionType.Exp,
                                scale=0.125,
                            )
                        for u in range(2):
                            ki = 2 * pj + u
                            m = ki - 4 * tcix
                            if m >= 0:
                                sx = m * 128
                                nc.vector.tensor_mul(
                                    wt[:, u, sx : sx + 128],
                                    wt[:, u, sx : sx + 128],
                                    mask_sb[:],
                                )
                        wts[(h, pj)] = wt
                        pend.append((h, pj))
                        tick()
                        if len(pend) > 3:
                            finish(pend.pop(0))
                while pend:
                    finish(pend.pop(0))

                for h in heads:
                    pb = (h % 2) * 64
                    ho = h // 2
                    if tail:
                        continue  # per-column-half chains emitted in the drain
                    # stage the AV psum to SBUF (frees the bank for the next
                    # head pair); transposed reciprocal via a DRAM bounce on
                    # the otherwise-idle scalar HW DGE queue
                    stage = rpool.tile([DH + 1, TCH], F32, tag="stage")
                    nc.vector.tensor_copy(stage[:], ps_av[h][0 : DH + 1, :])
                    d1 = dpool.tile([1, TCH], F32)
                    nc.sync.dma_start(d1[:], stage[DH : DH + 1, :])
                    rp = rpool.tile([128, 4], F32, tag="rp")
                    nc.sync.dma_start(
                        rp[:],
                        bass.AP(tensor=d1.tensor, offset=d1.offset, ap=[[4, 128], [1, 4]]),
                    )
                    rcp = rpool.tile([128, 4], F32, tag="rcp")
                    nc.vector.reciprocal(rcp[:], rp[:])
                    d2 = dpool.tile([1, TCH], F32)
                    nc.sync.dma_start(
                        bass.AP(tensor=d2.tensor, offset=d2.offset, ap=[[4, 128], [1, 4]]),
                        rcp[:],
                    )
                    rep = rpool.tile([64, TCH], F32, tag="rep")
                    nc.sync.dma_start(
                        rep[:],
                        bass.AP(tensor=d2.tensor, offset=d2.offset, ap=[[0, 64], [1, TCH]]),
                    )
                    nc.vector.tensor_mul(
                        attn_t[pb : pb + 64, ho, :], stage[0:DH, :], rep[:]
                    )

            while state["f"] < len(fillers):
                fillers[state["f"]]()
                state["f"] += 1

        for u in fc_units(n_chunks - 1):
            u()

    nc.compile()
    return nc


def _prep_core_inputs(x, w_qkv, b_qkv, w_fc, b_fc, core):
    b, g = core // 4, core % 4
    rq = slice(256 * g, 256 * g + 256)
    rk = slice(1024 + 256 * g, 1024 + 256 * g + 256)
    rv = slice(2048 + 256 * g, 2048 + 256 * g + 256)
    wcat = np.concatenate([w_qkv[rq], w_qkv[rk], w_qkv[rv]], axis=0)  # [768, 1024]
    bq, bk, bv = b_qkv[rq], b_qkv[rk], b_qkv[rv]
    import ml_dtypes

    bf16 = ml_dtypes.bfloat16
    # SBUF-tile layouts so every DMA moves full-line-rate contiguous runs:
    xT = x[b].T  # [C, T]
    xtl = xT.reshape(8, 128, T // TCH, TCH).transpose(2, 1, 0, 3)  # [tc, ci, co, t]
    wql = wcat.T.reshape(8, 128, 768).transpose(1, 0, 2)  # [ci, co, o]
    wqkl = wql[:, :, 0:512]
    wvl = wql[:, :, 512:768]
    wfcl = w_fc[:, 256 * g : 256 * g + 256].T.reshape(2, 128, 1024).transpose(1, 0, 2)
    return {
        "xtl": np.ascontiguousarray(xtl).astype(bf16),
        "wqkl": np.ascontiguousarray(wqkl).astype(bf16),
        "wvl": np.ascontiguousarray(wvl).astype(bf16),
        "bqk": np.ascontiguousarray(
            np.stack([bq[0:128], bq[128:256], bk[0:128], bk[128:256]], axis=1)
        ),
        "bv_rep": np.ascontiguousarray(np.broadcast_to(bv, (128, 256))),
        "wfcl": np.ascontiguousarray(wfcl).astype(bf16),
        "mask": np.triu(np.ones((128, 128), dtype=np.float32)).astype(bf16),
    }


def kernel(x, w_qkv, b_qkv, w_fc, b_fc):
    global LAST_RESULT
    x = np.asarray(x, dtype=np.float32)
    w_qkv = np.asarray(w_qkv, dtype=np.float32)
    b_qkv = np.asarray(b_qkv, dtype=np.float32)
    w_fc = np.asarray(w_fc, dtype=np.float32)
    b_fc = np.asarray(b_fc, dtype=np.float32)

    if "nc" not in _BUILD_CACHE:
        _BUILD_CACHE["nc"] = build()
    nc = _BUILD_CACHE["nc"]

    in_maps = [
        _prep_core_inputs(x, w_qkv, b_qkv, w_fc, b_fc, core) for core in range(8)
    ]
    res = run_bass_kernel_spmd(
        nc,
        in_maps,
        core_ids=list(range(8)),
        trace=bool(os.environ.get("MHA_TRACE")),
    )
    LAST_RESULT = res

    out = np.empty((B, T, C), dtype=np.float32)
    for b in range(B):
        acc = res.results[4 * b]["y"].astype(np.float32)
        for g in range(1, 4):
            acc = acc + res.results[4 * b + g]["y"].astype(np.float32)
        out[b] = acc + b_fc
    return out


# revision 42
# speedup vs baseline: 1.0006x; 1.0006x over previous
"""Causal multi-head attention block (QKV proj + causal softmax attention + out proj)
for Trainium2, sharded over 8 NeuronCores: data-parallel over batch (2), tensor-
parallel over heads (16 heads -> 4 per core).

Shapes (hardcoded): B=2, T=2048, C=1024, H=16, Dh=64.
Each core computes a partial output projection [T, C] for its 4 heads; the host
sums the 4 partials per batch and adds the fc bias.

Schedule: per t-chunk, the next chunk's QKV projection and the previous chunk's
FC are interleaved as PE filler units inside the attention pair stream, so the
tensor engine never waits on the softmax (exp on scalar, mask/normalize on
vector) dependency chain. Softmax denominators are reciprocal'd in a
transposed [128,4] layout (DRAM bounce on the sync HW DGE queue) mid-kernel;
the final head-pair instead uses an engine-only 32x32 stream-transpose chain
(the AV matmul replicates the denominator into 32 psum rows), emitted per
256-column half as soon as its last AV lands, so the kernel tail has no
DMA-semaphore latency in its critical path and the last FC units start as
early as the slice dependencies allow.
"""

import os

import numpy as np

import concourse.bass as bass
import concourse.tile as tile
from concourse import bacc, mybir
from concourse.bass_utils import run_bass_kernel_spmd

F32 = mybir.dt.float32
BF16 = mybir.dt.bfloat16

B = 2
T = 2048
C = 1024
H_PER_CORE = 4  # local heads per core
DH = 64
O_CORE = H_PER_CORE * DH  # 256 output channels per core (per q/k/v)

TCH = 512  # t-chunk size (free dim of most matmuls)
N_CHUNKS = T // TCH  # 4
KT = T // 128  # 16 k-tiles of 128

_BUILD_CACHE = {}
LAST_RESULT = None


def build(t=T):
    n_chunks = t // TCH
    nc = bacc.Bacc("TRN2", target_bir_lowering=False)

    xtl = nc.declare_dram_parameter("xtl", [t // TCH, 128, 8, TCH], BF16, isOutput=False)
    wql = nc.declare_dram_parameter("wql", [128, 8, 3 * O_CORE], BF16, isOutput=False)
    bqk = nc.declare_dram_parameter("bqk", [128, 4], F32, isOutput=False)
    bv_rep = nc.declare_dram_parameter("bv_rep", [128, O_CORE], F32, isOutput=False)
    wfcl = nc.declare_dram_parameter("wfcl", [128, 2, C], BF16, isOutput=False)
    mask = nc.declare_dram_parameter("mask", [128, 128], BF16, isOutput=False)
    y = nc.declare_dram_parameter("y", [t, C], BF16, isOutput=True)

    with (
        tile.TileContext(nc) as tc,
        tc.tile_pool(name="singles", bufs=1) as singles,
        tc.tile_pool(name="xpool", bufs=3) as xpool,
        tc.tile_pool(name="wtpool", bufs=8) as wtpool,
        tc.tile_pool(name="attnpool", bufs=3) as attnpool,
        tc.tile_pool(name="opool", bufs=4) as opool,
        tc.tile_pool(name="rpool", bufs=8) as rpool,
        tc.tile_pool(name="dpool", bufs=32, space="DRAM") as dpool,
        tc.tile_pool(name="mmps", bufs=2, space="PSUM") as mmps,
        tc.tile_pool(name="sps", bufs=2, space="PSUM") as sps,
        tc.tile_pool(name="avps", bufs=2, space="PSUM") as avps,
    ):
        # ---- persistent SBUF tensors ----
        wq_sb = singles.tile([128, 8, 3 * O_CORE], BF16)  # [ci, co, o]
        xt0 = xpool.tile([128, 8, TCH], BF16, tag="xt", name="xt0")
        bqk_sb = singles.tile([128, 4], F32)
        bv_sb = singles.tile([128, H_PER_CORE, DH], F32)
        mask_sb = singles.tile([128, 128], BF16)
        wfc_sb = singles.tile([128, 2, C], BF16)  # [p, ks, n]

        # Startup loads: host-pretiled (full-line-rate DMAs). The scalar HW
        # queue carries ONLY the wq loads and the sync queue leads with xt0,
        # so the first QKV matmuls' coalesced queue-counter waits cover
        # nothing but their own inputs.
        nc.scalar.dma_start(wq_sb[:, 0:4, :], wql[:, 0:4, :])
        nc.scalar.dma_start(xt0[:, 4:8, :], xtl[0, :, 4:8, :])
        nc.sync.dma_start(xt0[:, 0:4, :], xtl[0, :, 0:4, :])
        nc.sync.dma_start(wq_sb[:, 4:8, :], wql[:, 4:8, :])
        nc.sync.dma_start(bqk_sb[:], bqk[:])
        nc.sync.dma_start(mask_sb[:], mask[:])
        nc.sync.dma_start(bv_sb[:], bv_rep.rearrange("p (h d) -> p h d", h=H_PER_CORE))
        nc.sync.dma_start(wfc_sb[:], wfcl[:])

        qT_sb = singles.tile([128, 2, t], BF16)  # [dh + 64*(h%2), h//2, t]
        kT_sb = singles.tile([128, 2, t], BF16)
        v_sb = singles.tile([128, t // 128, H_PER_CORE, DH + 32], BF16)  # [k_in, kt, h, d|1s]
        nc.vector.memset(v_sb[:, :, :, DH : DH + 32], 1.0)  # 32 ones cols -> denom x32

        def qk_dst(i):
            return (qT_sb if i < 2 else kT_sb), i % 2

        # ---- chunk 0 QKV: co-major so matmuls start as soon as the first
        # co-slices of x and w land; 4 QK chains + 2 V chains accumulate
        # simultaneously across 6 psum banks.
        ps_qk2 = [
            sps.tile([128, 2, TCH], F32, tag="s", name="qks0"),
            sps.tile([128, 2, TCH], F32, tag="s", name="qks1"),
        ]
        # each interleaved accumulation chain must own a full PSUM bank:
        # interleaving two start/stop groups within one bank corrupts the
        # earlier group's start (observed on HW)
        ps_v = [
            mmps.tile([128, TCH], F32, tag="mm", name="vps0"),
            mmps.tile([128, TCH], F32, tag="mm", name="vps1"),
            avps.tile([128, TCH], F32, tag="av", name="vps2"),
            avps.tile([128, TCH], F32, tag="av", name="vps3"),
        ]
        for co in range(8):
            for i in range(4):
                nc.tensor.matmul(
                    ps_qk2[i // 2][:, i % 2, :],
                    wq_sb[:, co, i * 128 : (i + 1) * 128],
                    xt0[:, co, :],
                    start=(co == 0),
                    stop=(co == 7),
                    skip_group_check=True,
                )
            for tt in range(4):
                nc.tensor.matmul(
                    ps_v[tt][:, 0:256],
                    xt0[:, co, tt * 128 : (tt + 1) * 128],
                    wq_sb[:, co, 2 * O_CORE : 3 * O_CORE],
                    start=(co == 0),
                    stop=(co == 7),
                    skip_group_check=True,
                )
        for i in range(4):
            dst, half = qk_dst(i)
            nc.vector.tensor_scalar_add(
                dst[:, half, 0:TCH], ps_qk2[i // 2][:, i % 2, :], bqk_sb[:, i : i + 1]
            )
        for tt in range(4):
            nc.vector.tensor_add(
                v_sb[:, tt, :, 0:DH],
                ps_v[tt][:, 0:256].rearrange("p (h d) -> p h d", h=H_PER_CORE),
                bv_sb[:],
            )

        def load_xt(tcix):
            xt = xpool.tile([128, 8, TCH], BF16, tag="xt", name=f"xt{tcix}")
            nc.gpsimd.dma_start(xt[:, 0:4, :], xtl[tcix, :, 0:4, :])
            nc.gpsimd.dma_start(xt[:, 4:8, :], xtl[tcix, :, 4:8, :])
            return xt

        def qkv_units(tcix, xt):
            """Filler units (each a self-contained psum chain) for chunk tcix's
            QKV projection, interleaved into the previous chunk's attention."""
            ts0 = tcix * TCH

            def qk_group(i):
                ps = mmps.tile([128, TCH], F32, tag="mm", name="qkps")
                for co in range(8):
                    nc.tensor.matmul(
                        ps[:],
                        wq_sb[:, co, i * 128 : (i + 1) * 128],
                        xt[:, co, :],
                        start=(co == 0),
                        stop=(co == 7),
                        skip_group_check=True,
                    )
                dst, half = qk_dst(i)
                nc.vector.tensor_scalar_add(
                    dst[:, half, ts0 : ts0 + TCH], ps[:], bqk_sb[:, i : i + 1]
                )

            def v_group(j):
                ps = mmps.tile([128, TCH], F32, tag="mm", name="vps")
                for half in range(2):
                    tt = j * 2 + half
                    for co in range(8):
                        nc.tensor.matmul(
                            ps[:, half * 256 : (half + 1) * 256],
                            xt[:, co, tt * 128 : (tt + 1) * 128],
                            wq_sb[:, co, 2 * O_CORE : 3 * O_CORE],
                            start=(co == 0),
                            stop=(co == 7),
                            skip_group_check=True,
                        )
                kt0 = tcix * 4 + j * 2
                nc.vector.tensor_add(
                    v_sb[:, kt0 : kt0 + 2, :, 0:DH],
                    ps.rearrange("p (a h d) -> p a h d", a=2, h=H_PER_CORE),
                    bv_sb[:, None, :, :].to_broadcast((128, 2, H_PER_CORE, DH)),
                )

            units = [lambda i=i: qk_group(i) for i in range(4)]
            units += [lambda j=j: v_group(j) for j in range(2)]
            return units

        attn_tiles = {}

        def fc_units(tcix):
            """Filler units for chunk tcix's output projection."""
            ts0 = tcix * TCH
            at = attn_tiles[tcix]
            big = tcix == n_chunks - 1
            units = []
            for tt in range(4):
                for nn in range(2):
                    def u(tt=tt, nn=nn):
                        pool = avps if (big and (tt + nn) % 2 == 1) else mmps
                        tg = "av" if (big and (tt + nn) % 2 == 1) else "mm"
                        ps = pool.tile([128, TCH], F32, tag=tg, name="fcps")
                        for ks in range(2):
                            nc.tensor.matmul(
                                ps[:],
                                at[:, ks, tt * 128 : (tt + 1) * 128],
                                wfc_sb[:, ks, nn * TCH : (nn + 1) * TCH],
                                start=(ks == 0),
                                stop=(ks == 1),
                                skip_group_check=True,
                            )
                        ot = opool.tile([128, TCH], BF16, tag="o")
                        nc.vector.tensor_copy(ot[:], ps[:])
                        last = tcix == n_chunks - 1
                        eng = nc.scalar if last else nc.gpsimd
                        eng.dma_start(
                            y[ts0 + tt * 128 : ts0 + (tt + 1) * 128,
                              nn * TCH : (nn + 1) * TCH],
                            ot[:],
                        )
                    units.append(u)
            return units

        # ---- main loop: attention per chunk with filler interleaving ----
        for tcix in range(n_chunks):
            ts0 = tcix * TCH
            fillers = []
            if tcix + 1 < n_chunks:
                fillers += qkv_units(tcix + 1, load_xt(tcix + 1))
            if tcix >= 1:
                fillers += fc_units(tcix - 1)

            kimax = tcix * 4 + 3
            npairs = 2 * tcix + 2
            slots = 2 * npairs * 2
            state = {"s": 0, "f": 0}

            def tick():
                state["s"] += 1
                want = state["s"] * len(fillers) // slots
                while state["f"] < want:
                    fillers[state["f"]]()
                    state["f"] += 1

            attn_t = attnpool.tile([128, 2, TCH], BF16, tag="attn")
            attn_tiles[tcix] = attn_t
            for hp in range(2):
                heads = (2 * hp, 2 * hp + 1)
                ps_av = {}
                for h in heads:
                    ps_av[h] = avps.tile([128, TCH], F32, tag="av", name=f"av{h}")
                wts = {}
                pend = []

                tail = tcix == n_chunks - 1 and hp == 1
                mv = DH + 32 if tail else DH + 1

                def do_av(item):
                    h, pj = item
                    wt = wts[(h, pj)]
                    for u in range(2):
                        ki = 2 * pj + u
                        sx = max(0, (ki - 4 * tcix) * 128)
                        nc.tensor.matmul(
                            ps_av[h][0:mv, sx:TCH],
                            v_sb[:, ki, h, 0:mv],
                            wt[:, u, sx:TCH],
                            start=(ki == 0),
                            stop=(ki == kimax),
                            skip_group_check=True,
                        )

                def emit_half(h, c0):
                    pb = (h % 2) * 64
                    ho = h // 2
                    t1 = rpool.tile([32, 256], F32, tag="t1")
                    nc.vector.transpose(t1[:], ps_av[h][DH : DH + 32, c0 : c0 + 256])
                    t1v = t1.rearrange("p (j k) -> p j k", k=32)
                    rcp32 = rpool.tile([32, 8], F32, tag="rcp32")
                    nc.vector.reciprocal(rcp32[:], t1v[:, :, 0:1])
                    in2 = rpool.tile([32, 256], F32, tag="in2")
                    nc.vector.tensor_copy(
                        in2.rearrange("p (j k) -> p j k", k=32),
                        rcp32[:, :, None].to_broadcast((32, 8, 32)),
                    )
                    out2 = rpool.tile([32, 256], F32, tag="out2")
                    nc.vector.transpose(out2[:], in2[:])
                    nc.vector.tensor_mul(
                        attn_t[pb : pb + 32, ho, c0 : c0 + 256],
                        ps_av[h][0:32, c0 : c0 + 256],
                        out2[:],
                    )
                    nc.vector.tensor_mul(
                        attn_t[pb + 32 : pb + 64, ho, c0 : c0 + 256],
                        ps_av[h][32:DH, c0 : c0 + 256],
                        out2[:],
                    )

                def finish(item):
                    do_av(item)
                    if tail:
                        hh, pjj = item
                        if pjj == npairs - 2:
                            emit_half(hh, 0)
                        elif pjj == npairs - 1:
                            emit_half(hh, 256)

                for pj in range(npairs):
                    for h in heads:
                        pb = (h % 2) * 64
                        ho = h // 2
                        st = sps.tile([128, 2, TCH], F32, tag="s")
                        for u in range(2):
                            ki = 2 * pj + u
                            # columns left of the diagonal block are fully
                            # masked and never read downstream: skip them
                            m = ki - 4 * tcix
                            sx = 128 * m if m >= 1 else 0
                            nc.tensor.matmul(
                                st[:, u, sx:],
                                kT_sb[pb : pb + 64, ho, ki * 128 : (ki + 1) * 128],
                                qT_sb[pb : pb + 64, ho, ts0 + sx : ts0 + TCH],
                                start=True,
                                stop=True,
                                skip_group_check=True,
                            )
                        wt = wtpool.tile([128, 2, TCH], BF16, tag="wt")
                        m0 = 2 * pj - 4 * tcix  # block offset of the pair's first ki
                        if m0 >= 2:
                            # mostly-masked final pair: exp only the live columns
                            for u in range(2):
                                sx = (m0 + u) * 128
                                nc.scalar.activation(
                                    wt[:, u, sx:TCH],
                                    st[:, u, sx:TCH],
                                    mybir.ActivationFunctionType.Exp,
                                    scale=0.125,
                                )
                        else:
                            nc.scalar.activation(
                                wt[:],
                                st[:],
                                mybir.ActivationFunctionType.Exp,
                                scale=0.125,
                            )
                        for u in range(2):
                            ki = 2 * pj + u
                            m = ki - 4 * tcix
                            if m >= 0:
                                sx = m * 128
                                nc.vector.tensor_mul(
                                    wt[:, u, sx : sx + 128],
                                    wt[:, u, sx : sx + 128],
                                    mask_sb[:],
                                )
                        wts[(h, pj)] = wt
                        pend.append((h, pj))
                        tick()
                        if len(pend) > 3:
                            finish(pend.pop(0))
                while pend:
                    finish(pend.pop(0))

                for h in heads:
                    pb = (h % 2) * 64
                    ho = h // 2
                    if tail:
                        continue  # per-column-half chains emitted in the drain
                    # stage the AV psum to SBUF (frees the bank for the next
                    # head pair); transposed reciprocal via a DRAM bounce on
                    # the otherwise-idle scalar HW DGE queue
                    stage = rpool.tile([DH + 1, TCH], F32, tag="stage")
                    nc.vector.tensor_copy(stage[:], ps_av[h][0 : DH + 1, :])
                    d1 = dpool.tile([1, TCH], F32)
                    nc.sync.dma_start(d1[:], stage[DH : DH + 1, :])
                    rp = rpool.tile([128, 4], F32, tag="rp")
                    nc.sync.dma_start(
                        rp[:],
                        bass.AP(tensor=d1.tensor, offset=d1.offset, ap=[[4, 128], [1, 4]]),
                    )
                    rcp = rpool.tile([128, 4], F32, tag="rcp")
                    nc.vector.reciprocal(rcp[:], rp[:])
                    d2 = dpool.tile([1, TCH], F32)
                    nc.sync.dma_start(
                        bass.AP(tensor=d2.tensor, offset=d2.offset, ap=[[4, 128], [1, 4]]),
                        rcp[:],
                    )
                    rep = rpool.tile([64, TCH], F32, tag="rep")
                    nc.sync.dma_start(
                        rep[:],
                        bass.AP(tensor=d2.tensor, offset=d2.offset, ap=[[0, 64], [1, TCH]]),
                    )
                    nc.vector.tensor_mul(
                        attn_t[pb : pb + 64, ho, :], stage[0:DH, :], rep[:]
                    )

            while state["f"] < len(fillers):
                fillers[state["f"]]()
                state["f"] += 1

        for u in fc_units(n_chunks - 1):
            u()

    nc.compile()
    return nc


def _prep_core_inputs(x, w_qkv, b_qkv, w_fc, b_fc, core):
    b, g = core // 4, core % 4
    rq = slice(256 * g, 256 * g + 256)
    rk = slice(1024 + 256 * g, 1024 + 256 * g + 256)
    rv = slice(2048 + 256 * g, 2048 + 256 * g + 256)
    wcat = np.concatenate([w_qkv[rq], w_qkv[rk], w_qkv[rv]], axis=0)  # [768, 1024]
    bq, bk, bv = b_qkv[rq], b_qkv[rk], b_qkv[rv]
    import ml_dtypes

    bf16 = ml_dtypes.bfloat16
    # SBUF-tile layouts so every DMA moves full-line-rate contiguous runs:
    xT = x[b].T  # [C, T]
    xtl = xT.reshape(8, 128, T // TCH, TCH).transpose(2, 1, 0, 3)  # [tc, ci, co, t]
    wql = wcat.T.reshape(8, 128, 768).transpose(1, 0, 2)  # [ci, co, o]
    wfcl = w_fc[:, 256 * g : 256 * g + 256].T.reshape(2, 128, 1024).transpose(1, 0, 2)
    return {
        "xtl": np.ascontiguousarray(xtl).astype(bf16),
        "wql": np.ascontiguousarray(wql).astype(bf16),
        "bqk": np.ascontiguousarray(
            np.stack([bq[0:128], bq[128:256], bk[0:128], bk[128:256]], axis=1)
        ),
        "bv_rep": np.ascontiguousarray(np.broadcast_to(bv, (128, 256))),
        "wfcl": np.ascontiguousarray(wfcl).astype(bf16),
        "mask": np.triu(np.ones((128, 128), dtype=np.float32)).astype(bf16),
    }


def kernel(x, w_qkv, b_qkv, w_fc, b_fc):
    global LAST_RESULT
    x = np.asarray(x, dtype=np.float32)
    w_qkv = np.asarray(w_qkv, dtype=np.float32)
    b_qkv = np.asarray(b_qkv, dtype=np.float32)
    w_fc = np.asarray(w_fc, dtype=np.float32)
    b_fc = np.asarray(b_fc, dtype=np.float32)

    if "nc" not in _BUILD_CACHE:
        _BUILD_CACHE["nc"] = build()
    nc = _BUILD_CACHE["nc"]

    in_maps = [
        _prep_core_inputs(x, w_qkv, b_qkv, w_fc, b_fc, core) for core in range(8)
    ]
    res = run_bass_kernel_spmd(
        nc,
        in_maps,
        core_ids=list(range(8)),
        trace=bool(os.environ.get("MHA_TRACE")),
    )
    LAST_RESULT = res

    out = np.empty((B, T, C), dtype=np.float32)
    for b in range(B):
        acc = res.results[4 * b]["y"].astype(np.float32)
        for g in range(1, 4):
            acc = acc + res.results[4 * b + g]["y"].astype(np.float32)
        out[b] = acc + b_fc
    return out


# revision 43
# speedup vs baseline: 1.0029x; 1.0023x over previous
"""Causal multi-head attention block (QKV proj + causal softmax attention + out proj)
for Trainium2, sharded over 8 NeuronCores: data-parallel over batch (2), tensor-
parallel over heads (16 heads -> 4 per core).

Shapes (hardcoded): B=2, T=2048, C=1024, H=16, Dh=64.
Each core computes a partial output projection [T, C] for its 4 heads; the host
sums the 4 partials per batch and adds the fc bias.

Schedule: per t-chunk, the next chunk's QKV projection and the previous chunk's
FC are interleaved as PE filler units inside the attention pair stream, so the
tensor engine never waits on the softmax (exp on scalar, mask/normalize on
vector) dependency chain. Softmax denominators are reciprocal'd in a
transposed [128,4] layout (DRAM bounce on the sync HW DGE queue) mid-kernel;
the final head-pair instead uses an engine-only 32x32 stream-transpose chain
(the AV matmul replicates the denominator into 32 psum rows), emitted per
256-column half as soon as its last AV lands, so the kernel tail has no
DMA-semaphore latency in its critical path and the last FC units start as
early as the slice dependencies allow.
"""

import os

import numpy as np

import concourse.bass as bass
import concourse.tile as tile
from concourse import bacc, mybir
from concourse.bass_utils import run_bass_kernel_spmd

F32 = mybir.dt.float32
BF16 = mybir.dt.bfloat16

B = 2
T = 2048
C = 1024
H_PER_CORE = 4  # local heads per core
DH = 64
O_CORE = H_PER_CORE * DH  # 256 output channels per core (per q/k/v)

TCH = 512  # t-chunk size (free dim of most matmuls)
N_CHUNKS = T // TCH  # 4
KT = T // 128  # 16 k-tiles of 128

_BUILD_CACHE = {}
LAST_RESULT = None


def build(t=T):
    n_chunks = t // TCH
    nc = bacc.Bacc("TRN2", target_bir_lowering=False)

    xtl = nc.declare_dram_parameter("xtl", [t // TCH, 128, 8, TCH], BF16, isOutput=False)
    wql = nc.declare_dram_parameter("wql", [128, 8, 3 * O_CORE], BF16, isOutput=False)
    bqk = nc.declare_dram_parameter("bqk", [128, 4], F32, isOutput=False)
    bv_rep = nc.declare_dram_parameter("bv_rep", [128, O_CORE], F32, isOutput=False)
    wfcl = nc.declare_dram_parameter("wfcl", [128, 2, C], BF16, isOutput=False)
    mask = nc.declare_dram_parameter("mask", [128, 128], BF16, isOutput=False)
    y = nc.declare_dram_parameter("y", [t, C], BF16, isOutput=True)

    with (
        tile.TileContext(nc) as tc,
        tc.tile_pool(name="singles", bufs=1) as singles,
        tc.tile_pool(name="xpool", bufs=3) as xpool,
        tc.tile_pool(name="wtpool", bufs=8) as wtpool,
        tc.tile_pool(name="attnpool", bufs=3) as attnpool,
        tc.tile_pool(name="opool", bufs=4) as opool,
        tc.tile_pool(name="rpool", bufs=8) as rpool,
        tc.tile_pool(name="dpool", bufs=32, space="DRAM") as dpool,
        tc.tile_pool(name="mmps", bufs=2, space="PSUM") as mmps,
        tc.tile_pool(name="sps", bufs=2, space="PSUM") as sps,
        tc.tile_pool(name="avps", bufs=2, space="PSUM") as avps,
    ):
        # ---- persistent SBUF tensors ----
        wq_sb = singles.tile([128, 8, 3 * O_CORE], BF16)  # [ci, co, o]
        xt0 = xpool.tile([128, 8, TCH], BF16, tag="xt", name="xt0")
        bqk_sb = singles.tile([128, 4], F32)
        bv_sb = singles.tile([128, H_PER_CORE, DH], F32)
        mask_sb = singles.tile([128, 128], BF16)
        wfc_sb = singles.tile([128, 2, C], BF16)  # [p, ks, n]

        # Startup loads: host-pretiled (full-line-rate DMAs). The scalar HW
        # queue carries ONLY the wq loads and the sync queue leads with xt0,
        # so the first QKV matmuls' coalesced queue-counter waits cover
        # nothing but their own inputs.
        nc.scalar.dma_start(wq_sb[:, 0:4, :], wql[:, 0:4, :])
        nc.scalar.dma_start(xt0[:, 4:8, :], xtl[0, :, 4:8, :])
        nc.sync.dma_start(xt0[:, 0:4, :], xtl[0, :, 0:4, :])
        nc.sync.dma_start(wq_sb[:, 4:8, :], wql[:, 4:8, :])
        nc.sync.dma_start(bqk_sb[:], bqk[:])
        nc.sync.dma_start(mask_sb[:], mask[:])
        nc.sync.dma_start(bv_sb[:], bv_rep.rearrange("p (h d) -> p h d", h=H_PER_CORE))
        nc.sync.dma_start(wfc_sb[:], wfcl[:])

        qT_sb = singles.tile([128, 2, t], BF16)  # [dh + 64*(h%2), h//2, t]
        kT_sb = singles.tile([128, 2, t], BF16)
        v_sb = singles.tile([128, t // 128, H_PER_CORE, DH + 32], BF16)  # [k_in, kt, h, d|1s]
        nc.vector.memset(v_sb[:, :, :, DH : DH + 32], 1.0)  # 32 ones cols -> denom x32

        def qk_dst(i):
            return (qT_sb if i < 2 else kT_sb), i % 2

        # ---- chunk 0 QKV: co-major so matmuls start as soon as the first
        # co-slices of x and w land; 4 QK chains + 2 V chains accumulate
        # simultaneously across 6 psum banks.
        ps_qk2 = [
            sps.tile([128, 2, TCH], F32, tag="s", name="qks0"),
            sps.tile([128, 2, TCH], F32, tag="s", name="qks1"),
        ]
        # each interleaved accumulation chain must own a full PSUM bank:
        # interleaving two start/stop groups within one bank corrupts the
        # earlier group's start (observed on HW)
        ps_v = [
            mmps.tile([128, TCH], F32, tag="mm", name="vps0"),
            mmps.tile([128, TCH], F32, tag="mm", name="vps1"),
            avps.tile([128, TCH], F32, tag="av", name="vps2"),
            avps.tile([128, TCH], F32, tag="av", name="vps3"),
        ]
        for co in range(8):
            for i in range(4):
                nc.tensor.matmul(
                    ps_qk2[i // 2][:, i % 2, :],
                    wq_sb[:, co, i * 128 : (i + 1) * 128],
                    xt0[:, co, :],
                    start=(co == 0),
                    stop=(co == 7),
                    skip_group_check=True,
                )
            for tt in range(4):
                nc.tensor.matmul(
                    ps_v[tt][:, 0:256],
                    xt0[:, co, tt * 128 : (tt + 1) * 128],
                    wq_sb[:, co, 2 * O_CORE : 3 * O_CORE],
                    start=(co == 0),
                    stop=(co == 7),
                    skip_group_check=True,
                )
        for i in range(4):
            dst, half = qk_dst(i)
            nc.vector.tensor_scalar_add(
                dst[:, half, 0:TCH], ps_qk2[i // 2][:, i % 2, :], bqk_sb[:, i : i + 1]
            )
        for tt in range(4):
            nc.vector.tensor_add(
                v_sb[:, tt, :, 0:DH],
                ps_v[tt][:, 0:256].rearrange("p (h d) -> p h d", h=H_PER_CORE),
                bv_sb[:],
            )

        def load_xt(tcix):
            xt = xpool.tile([128, 8, TCH], BF16, tag="xt", name=f"xt{tcix}")
            nc.gpsimd.dma_start(xt[:, 0:4, :], xtl[tcix, :, 0:4, :])
            nc.gpsimd.dma_start(xt[:, 4:8, :], xtl[tcix, :, 4:8, :])
            return xt

        def qkv_units(tcix, xt):
            """Filler units (each a self-contained psum chain) for chunk tcix's
            QKV projection, interleaved into the previous chunk's attention."""
            ts0 = tcix * TCH

            def qk_group(i):
                ps = mmps.tile([128, TCH], F32, tag="mm", name="qkps")
                for co in range(8):
                    nc.tensor.matmul(
                        ps[:],
                        wq_sb[:, co, i * 128 : (i + 1) * 128],
                        xt[:, co, :],
                        start=(co == 0),
                        stop=(co == 7),
                        skip_group_check=True,
                    )
                dst, half = qk_dst(i)
                nc.vector.tensor_scalar_add(
                    dst[:, half, ts0 : ts0 + TCH], ps[:], bqk_sb[:, i : i + 1]
                )

            def v_group(j):
                ps = mmps.tile([128, TCH], F32, tag="mm", name="vps")
                for half in range(2):
                    tt = j * 2 + half
                    for co in range(8):
                        nc.tensor.matmul(
                            ps[:, half * 256 : (half + 1) * 256],
                            xt[:, co, tt * 128 : (tt + 1) * 128],
                            wq_sb[:, co, 2 * O_CORE : 3 * O_CORE],
                            start=(co == 0),
                            stop=(co == 7),
                            skip_group_check=True,
                        )
                kt0 = tcix * 4 + j * 2
                nc.vector.tensor_add(
                    v_sb[:, kt0 : kt0 + 2, :, 0:DH],
                    ps.rearrange("p (a h d) -> p a h d", a=2, h=H_PER_CORE),
                    bv_sb[:, None, :, :].to_broadcast((128, 2, H_PER_CORE, DH)),
                )

            units = [lambda i=i: qk_group(i) for i in range(4)]
            units += [lambda j=j: v_group(j) for j in range(2)]
            return units

        attn_tiles = {}

        def fc_units(tcix):
            """Filler units for chunk tcix's output projection."""
            ts0 = tcix * TCH
            at = attn_tiles[tcix]
            big = tcix == n_chunks - 1
            units = []
            for tt in range(4):
                for nn in range(2):
                    def u(tt=tt, nn=nn):
                        pool = avps if (big and (tt + nn) % 2 == 1) else mmps
                        tg = "av" if (big and (tt + nn) % 2 == 1) else "mm"
                        ps = pool.tile([128, TCH], F32, tag=tg, name="fcps")
                        for ks in range(2):
                            nc.tensor.matmul(
                                ps[:],
                                at[:, ks, tt * 128 : (tt + 1) * 128],
                                wfc_sb[:, ks, nn * TCH : (nn + 1) * TCH],
                                start=(ks == 0),
                                stop=(ks == 1),
                                skip_group_check=True,
                            )
                        ot = opool.tile([128, TCH], BF16, tag="o")
                        nc.vector.tensor_copy(ot[:], ps[:])
                        last = tcix == n_chunks - 1
                        eng = nc.scalar if last else nc.gpsimd
                        eng.dma_start(
                            y[ts0 + tt * 128 : ts0 + (tt + 1) * 128,
                              nn * TCH : (nn + 1) * TCH],
                            ot[:],
                        )
                    units.append(u)
            return units

        # ---- main loop: attention per chunk with filler interleaving ----
        for tcix in range(n_chunks):
            ts0 = tcix * TCH
            fillers = []
            if tcix + 1 < n_chunks:
                fillers += qkv_units(tcix + 1, load_xt(tcix + 1))
            if tcix >= 1:
                fillers += fc_units(tcix - 1)

            kimax = tcix * 4 + 3
            npairs = 2 * tcix + 2
            slots = 2 * npairs * 2
            state = {"s": 0, "f": 0}

            def tick():
                state["s"] += 1
                want = state["s"] * len(fillers) // slots
                while state["f"] < want:
                    fillers[state["f"]]()
                    state["f"] += 1

            attn_t = attnpool.tile([128, 2, TCH], BF16, tag="attn")
            attn_tiles[tcix] = attn_t
            for hp in range(2):
                heads = (2 * hp, 2 * hp + 1)
                ps_av = {}
                for h in heads:
                    ps_av[h] = avps.tile([128, TCH], F32, tag="av", name=f"av{h}")
                wts = {}
                pend = []

                tail = tcix == n_chunks - 1 and hp == 1
                mv = DH + 32 if tail else DH + 1

                def do_av(item):
                    h, pj = item
                    wt = wts[(h, pj)]
                    for u in range(2):
                        ki = 2 * pj + u
                        sx = max(0, (ki - 4 * tcix) * 128)
                        nc.tensor.matmul(
                            ps_av[h][0:mv, sx:TCH],
                            v_sb[:, ki, h, 0:mv],
                            wt[:, u, sx:TCH],
                            start=(ki == 0),
                            stop=(ki == kimax),
                            skip_group_check=True,
                        )

                def emit_half(h, c0):
                    pb = (h % 2) * 64
                    ho = h // 2
                    t1 = rpool.tile([32, 256], F32, tag="t1")
                    nc.vector.transpose(t1[:], ps_av[h][DH : DH + 32, c0 : c0 + 256])
                    t1v = t1.rearrange("p (j k) -> p j k", k=32)
                    rcp32 = rpool.tile([32, 8], F32, tag="rcp32")
                    nc.vector.reciprocal(rcp32[:], t1v[:, :, 0:1])
                    in2 = rpool.tile([32, 256], F32, tag="in2")
                    nc.vector.tensor_copy(
                        in2.rearrange("p (j k) -> p j k", k=32),
                        rcp32[:, :, None].to_broadcast((32, 8, 32)),
                    )
                    out2 = rpool.tile([32, 256], F32, tag="out2")
                    nc.vector.transpose(out2[:], in2[:])
                    nc.vector.tensor_mul(
                        attn_t[pb : pb + 32, ho, c0 : c0 + 256],
                        ps_av[h][0:32, c0 : c0 + 256],
                        out2[:],
                    )
                    nc.vector.tensor_mul(
                        attn_t[pb + 32 : pb + 64, ho, c0 : c0 + 256],
                        ps_av[h][32:DH, c0 : c0 + 256],
                        out2[:],
                    )

                def finish(item):
                    do_av(item)
                    if tail:
                        hh, pjj = item
                        if pjj == npairs - 2:
                            emit_half(hh, 0)
                        elif pjj == npairs - 1:
                            emit_half(hh, 256)

                for pj in range(npairs):
                    for h in heads:
                        pb = (h % 2) * 64
                        ho = h // 2
                        st = sps.tile([128, 2, TCH], F32, tag="s")
                        for u in range(2):
                            ki = 2 * pj + u
                            # columns left of the diagonal block are fully
                            # masked and never read downstream: skip them
                            m = ki - 4 * tcix
                            sx = 128 * m if m >= 1 else 0
                            nc.tensor.matmul(
                                st[:, u, sx:],
                                kT_sb[pb : pb + 64, ho, ki * 128 : (ki + 1) * 128],
                                qT_sb[pb : pb + 64, ho, ts0 + sx : ts0 + TCH],
                                start=True,
                                stop=True,
                                skip_group_check=True,
                            )
                        wt = wtpool.tile([128, 2, TCH], BF16, tag="wt")
                        m0 = 2 * pj - 4 * tcix  # block offset of the pair's first ki
                        if m0 >= 2:
                            # mostly-masked final pair: exp only the live columns
                            for u in range(2):
                                sx = (m0 + u) * 128
                                nc.scalar.activation(
                                    wt[:, u, sx:TCH],
                                    st[:, u, sx:TCH],
                                    mybir.ActivationFunctionType.Exp,
                                    scale=0.125,
                                )
                        else:
                            nc.scalar.activation(
                                wt[:],
                                st[:],
                                mybir.ActivationFunctionType.Exp,
                                scale=0.125,
                            )
                        for u in range(2):
                            ki = 2 * pj + u
                            m = ki - 4 * tcix
                            if m >= 0:
                                sx = m * 128
                                nc.vector.tensor_mul(
                                    wt[:, u, sx : sx + 128],
                                    wt[:, u, sx : sx + 128],
                                    mask_sb[:],
                                )
                        wts[(h, pj)] = wt
                        pend.append((h, pj))
                        if len(pend) > 3:
                            finish(pend.pop(0))
                        tick()
                while pend:
                    finish(pend.pop(0))

                for h in heads:
                    pb = (h % 2) * 64
                    ho = h // 2
                    if tail:
                        continue  # per-column-half chains emitted in the drain
                    # stage the AV psum to SBUF (frees the bank for the next
                    # head pair); transposed reciprocal via a DRAM bounce on
                    # the otherwise-idle scalar HW DGE queue
                    stage = rpool.tile([DH + 1, TCH], F32, tag="stage")
                    nc.vector.tensor_copy(stage[:], ps_av[h][0 : DH + 1, :])
                    d1 = dpool.tile([1, TCH], F32)
                    nc.sync.dma_start(d1[:], stage[DH : DH + 1, :])
                    rp = rpool.tile([128, 4], F32, tag="rp")
                    nc.sync.dma_start(
                        rp[:],
                        bass.AP(tensor=d1.tensor, offset=d1.offset, ap=[[4, 128], [1, 4]]),
                    )
                    rcp = rpool.tile([128, 4], F32, tag="rcp")
                    nc.vector.reciprocal(rcp[:], rp[:])
                    d2 = dpool.tile([1, TCH], F32)
                    nc.sync.dma_start(
                        bass.AP(tensor=d2.tensor, offset=d2.offset, ap=[[4, 128], [1, 4]]),
                        rcp[:],
                    )
                    rep = rpool.tile([64, TCH], F32, tag="rep")
                    nc.sync.dma_start(
                        rep[:],
                        bass.AP(tensor=d2.tensor, offset=d2.offset, ap=[[0, 64], [1, TCH]]),
                    )
                    nc.vector.tensor_mul(
                        attn_t[pb : pb + 64, ho, :], stage[0:DH, :], rep[:]
                    )

            while state["f"] < len(fillers):
                fillers[state["f"]]()
                state["f"] += 1

        for u in fc_units(n_chunks - 1):
            u()

    nc.compile()
    return nc


def _prep_core_inputs(x, w_qkv, b_qkv, w_fc, b_fc, core):
    b, g = core // 4, core % 4
    rq = slice(256 * g, 256 * g + 256)
    rk = slice(1024 + 256 * g, 1024 + 256 * g + 256)
    rv = slice(2048 + 256 * g, 2048 + 256 * g + 256)
    wcat = np.concatenate([w_qkv[rq], w_qkv[rk], w_qkv[rv]], axis=0)  # [768, 1024]
    bq, bk, bv = b_qkv[rq], b_qkv[rk], b_qkv[rv]
    import ml_dtypes

    bf16 = ml_dtypes.bfloat16
    # SBUF-tile layouts so every DMA moves full-line-rate contiguous runs:
    xT = x[b].T  # [C, T]
    xtl = xT.reshape(8, 128, T // TCH, TCH).transpose(2, 1, 0, 3)  # [tc, ci, co, t]
    wql = wcat.T.reshape(8, 128, 768).transpose(1, 0, 2)  # [ci, co, o]
    wfcl = w_fc[:, 256 * g : 256 * g + 256].T.reshape(2, 128, 1024).transpose(1, 0, 2)
    return {
        "xtl": np.ascontiguousarray(xtl).astype(bf16),
        "wql": np.ascontiguousarray(wql).astype(bf16),
        "bqk": np.ascontiguousarray(
            np.stack([bq[0:128], bq[128:256], bk[0:128], bk[128:256]], axis=1)
        ),
        "bv_rep": np.ascontiguousarray(np.broadcast_to(bv, (128, 256))),
        "wfcl": np.ascontiguousarray(wfcl).astype(bf16),
        "mask": np.triu(np.ones((128, 128), dtype=np.float32)).astype(bf16),
    }


def kernel(x, w_qkv, b_qkv, w_fc, b_fc):
    global LAST_RESULT
    x = np.asarray(x, dtype=np.float32)
    w_qkv = np.asarray(w_qkv, dtype=np.float32)
    b_qkv = np.asarray(b_qkv, dtype=np.float32)
    w_fc = np.asarray(w_fc, dtype=np.float32)
    b_fc = np.asarray(b_fc, dtype=np.float32)

    if "nc" not in _BUILD_CACHE:
        _BUILD_CACHE["nc"] = build()
    nc = _BUILD_CACHE["nc"]

    in_maps = [
        _prep_core_inputs(x, w_qkv, b_qkv, w_fc, b_fc, core) for core in range(8)
    ]
    res = run_bass_kernel_spmd(
        nc,
        in_maps,
        core_ids=list(range(8)),
        trace=bool(os.environ.get("MHA_TRACE")),
    )
    LAST_RESULT = res

    out = np.empty((B, T, C), dtype=np.float32)
    for b in range(B):
        acc = res.results[4 * b]["y"].astype(np.float32)
        for g in range(1, 4):
            acc = acc + res.results[4 * b + g]["y"].astype(np.float32)
        out[b] = acc + b_fc
    return out


# revision 44
# speedup vs baseline: 1.0571x; 1.0540x over previous
"""Causal multi-head attention block (QKV proj + causal softmax attention + out proj)
for Trainium2, sharded over 8 NeuronCores: data-parallel over batch (2), tensor-
parallel over heads (16 heads -> 4 per core).

Shapes (hardcoded): B=2, T=2048, C=1024, H=16, Dh=64.
Each core computes a partial output projection [T, C] for its 4 heads; the host
sums the 4 partials per batch and adds the fc bias.

Schedule: per t-chunk, the next chunk's QKV projection and the previous chunk's
FC are interleaved as PE filler units inside the attention pair stream, so the
tensor engine never waits on the softmax (exp on scalar, mask/normalize on
vector) dependency chain. Softmax denominators are reciprocal'd in a
transposed [128,4] layout (DRAM bounce on the sync HW DGE queue) mid-kernel;
the final head-pair instead uses an engine-only 32x32 stream-transpose chain
(the AV matmul replicates the denominator into 32 psum rows), emitted per
256-column half as soon as its last AV lands, so the kernel tail has no
DMA-semaphore latency in its critical path and the last FC units start as
early as the slice dependencies allow.
"""

import os

import numpy as np

import concourse.bass as bass
import concourse.tile as tile
from concourse import bacc, mybir
from concourse.bass_utils import run_bass_kernel_spmd

F32 = mybir.dt.float32
BF16 = mybir.dt.bfloat16

B = 2
T = 2048
C = 1024
H_PER_CORE = 4  # local heads per core
DH = 64
O_CORE = H_PER_CORE * DH  # 256 output channels per core (per q/k/v)

TCH = 512  # t-chunk size (free dim of most matmuls)
N_CHUNKS = T // TCH  # 4
KT = T // 128  # 16 k-tiles of 128

_BUILD_CACHE = {}
LAST_RESULT = None


def build(t=T):
    n_chunks = t // TCH
    nc = bacc.Bacc("TRN2", target_bir_lowering=False)

    xtl = nc.declare_dram_parameter("xtl", [t // TCH, 128, 8, TCH], BF16, isOutput=False)
    wql = nc.declare_dram_parameter("wql", [128, 8, 3 * O_CORE], BF16, isOutput=False)
    bqk = nc.declare_dram_parameter("bqk", [128, 4], F32, isOutput=False)
    bv_rep = nc.declare_dram_parameter("bv_rep", [128, O_CORE], F32, isOutput=False)
    wfcl = nc.declare_dram_parameter("wfcl", [128, 2, C], BF16, isOutput=False)
    mask = nc.declare_dram_parameter("mask", [128, 128], BF16, isOutput=False)
    y = nc.declare_dram_parameter("y", [t, C], BF16, isOutput=True)

    with (
        tile.TileContext(nc) as tc,
        tc.tile_pool(name="singles", bufs=1) as singles,
        tc.tile_pool(name="xpool", bufs=3) as xpool,
        tc.tile_pool(name="wtpool", bufs=8) as wtpool,
        tc.tile_pool(name="attnpool", bufs=3) as attnpool,
        tc.tile_pool(name="opool", bufs=4) as opool,
        tc.tile_pool(name="rpool", bufs=8) as rpool,
        tc.tile_pool(name="dpool", bufs=32, space="DRAM") as dpool,
        tc.tile_pool(name="mmps", bufs=2, space="PSUM") as mmps,
        tc.tile_pool(name="sps", bufs=2, space="PSUM") as sps,
        tc.tile_pool(name="avps", bufs=2, space="PSUM") as avps,
    ):
        # ---- persistent SBUF tensors ----
        wq_sb = singles.tile([128, 8, 3 * O_CORE], BF16)  # [ci, co, o]
        xt0 = xpool.tile([128, 8, TCH], BF16, tag="xt", name="xt0")
        bqk_sb = singles.tile([128, 4], F32)
        bv_sb = singles.tile([128, H_PER_CORE, DH], F32)
        mask_sb = singles.tile([128, 128], BF16)
        wfc_sb = singles.tile([128, 2, C], BF16)  # [p, ks, n]

        # Startup loads: host-pretiled (full-line-rate DMAs). The scalar HW
        # queue carries ONLY the wq loads and the sync queue leads with xt0,
        # so the first QKV matmuls' coalesced queue-counter waits cover
        # nothing but their own inputs.
        nc.scalar.dma_start(wq_sb[:, 0:4, :], wql[:, 0:4, :])
        nc.scalar.dma_start(xt0[:, 4:8, :], xtl[0, :, 4:8, :])
        nc.sync.dma_start(xt0[:, 0:4, :], xtl[0, :, 0:4, :])
        nc.sync.dma_start(wq_sb[:, 4:8, :], wql[:, 4:8, :])
        nc.sync.dma_start(bqk_sb[:], bqk[:])
        nc.sync.dma_start(mask_sb[:], mask[:])
        nc.sync.dma_start(bv_sb[:], bv_rep.rearrange("p (h d) -> p h d", h=H_PER_CORE))
        nc.sync.dma_start(wfc_sb[:], wfcl[:])

        qT_sb = singles.tile([128, 2, t], BF16)  # [dh + 64*(h%2), h//2, t]
        kT_sb = singles.tile([128, 2, t], BF16)
        v_sb = singles.tile([128, t // 128, H_PER_CORE, DH + 32], BF16)  # [k_in, kt, h, d|1s]
        nc.vector.memset(v_sb[:, :, :, DH : DH + 32], 1.0)  # 32 ones cols -> denom x32

        def qk_dst(i):
            return (qT_sb if i < 2 else kT_sb), i % 2

        # ---- chunk 0 QKV: co-major so matmuls start as soon as the first
        # co-slices of x and w land; 4 QK chains + 2 V chains accumulate
        # simultaneously across 6 psum banks.
        ps_qk2 = [
            sps.tile([128, 2, TCH], F32, tag="s", name="qks0"),
            sps.tile([128, 2, TCH], F32, tag="s", name="qks1"),
        ]
        # each interleaved accumulation chain must own a full PSUM bank:
        # interleaving two start/stop groups within one bank corrupts the
        # earlier group's start (observed on HW)
        ps_v = [
            mmps.tile([128, TCH], F32, tag="mm", name="vps0"),
            mmps.tile([128, TCH], F32, tag="mm", name="vps1"),
            avps.tile([128, TCH], F32, tag="av", name="vps2"),
            avps.tile([128, TCH], F32, tag="av", name="vps3"),
        ]
        for co in range(8):
            for i in range(4):
                nc.tensor.matmul(
                    ps_qk2[i // 2][:, i % 2, :],
                    wq_sb[:, co, i * 128 : (i + 1) * 128],
                    xt0[:, co, :],
                    start=(co == 0),
                    stop=(co == 7),
                    skip_group_check=True,
                )
            for tt in range(4):
                nc.tensor.matmul(
                    ps_v[tt][:, 0:256],
                    xt0[:, co, tt * 128 : (tt + 1) * 128],
                    wq_sb[:, co, 2 * O_CORE : 3 * O_CORE],
                    start=(co == 0),
                    stop=(co == 7),
                    skip_group_check=True,
                )
        for i in range(4):
            dst, half = qk_dst(i)
            nc.vector.tensor_scalar_add(
                dst[:, half, 0:TCH], ps_qk2[i // 2][:, i % 2, :], bqk_sb[:, i : i + 1]
            )
        for tt in range(4):
            nc.vector.tensor_add(
                v_sb[:, tt, :, 0:DH],
                ps_v[tt][:, 0:256].rearrange("p (h d) -> p h d", h=H_PER_CORE),
                bv_sb[:],
            )

        def load_xt(tcix):
            xt = xpool.tile([128, 8, TCH], BF16, tag="xt", name=f"xt{tcix}")
            nc.gpsimd.dma_start(xt[:, 0:4, :], xtl[tcix, :, 0:4, :])
            nc.gpsimd.dma_start(xt[:, 4:8, :], xtl[tcix, :, 4:8, :])
            return xt

        def qkv_units(tcix, xt):
            """Filler units (each a self-contained psum chain) for chunk tcix's
            QKV projection, interleaved into the previous chunk's attention."""
            ts0 = tcix * TCH

            def qk_group(i):
                ps = mmps.tile([128, TCH], F32, tag="mm", name="qkps")
                for co in range(8):
                    nc.tensor.matmul(
                        ps[:],
                        wq_sb[:, co, i * 128 : (i + 1) * 128],
                        xt[:, co, :],
                        start=(co == 0),
                        stop=(co == 7),
                        skip_group_check=True,
                    )
                dst, half = qk_dst(i)
                nc.vector.tensor_scalar_add(
                    dst[:, half, ts0 : ts0 + TCH], ps[:], bqk_sb[:, i : i + 1]
                )

            def v_group(j):
                ps = mmps.tile([128, TCH], F32, tag="mm", name="vps")
                for half in range(2):
                    tt = j * 2 + half
                    for co in range(8):
                        nc.tensor.matmul(
                            ps[:, half * 256 : (half + 1) * 256],
                            xt[:, co, tt * 128 : (tt + 1) * 128],
                            wq_sb[:, co, 2 * O_CORE : 3 * O_CORE],
                            start=(co == 0),
                            stop=(co == 7),
                            skip_group_check=True,
                        )
                kt0 = tcix * 4 + j * 2
                nc.vector.tensor_add(
                    v_sb[:, kt0 : kt0 + 2, :, 0:DH],
                    ps.rearrange("p (a h d) -> p a h d", a=2, h=H_PER_CORE),
                    bv_sb[:, None, :, :].to_broadcast((128, 2, H_PER_CORE, DH)),
                )

            units = [lambda i=i: qk_group(i) for i in range(4)]
            units += [lambda j=j: v_group(j) for j in range(2)]
            return units

        attn_tiles = {}

        def fc_units(tcix):
            """Filler units for chunk tcix's output projection."""
            ts0 = tcix * TCH
            at = attn_tiles[tcix]
            big = tcix == n_chunks - 1
            units = []
            for tt in range(4):
                for nn in range(2):
                    def u(tt=tt, nn=nn):
                        pool = avps if (big and (tt + nn) % 2 == 1) else mmps
                        tg = "av" if (big and (tt + nn) % 2 == 1) else "mm"
                        ps = pool.tile([128, TCH], F32, tag=tg, name="fcps")
                        for ks in range(2):
                            nc.tensor.matmul(
                                ps[:],
                                at[:, ks, tt * 128 : (tt + 1) * 128],
                                wfc_sb[:, ks, nn * TCH : (nn + 1) * TCH],
                                start=(ks == 0),
                                stop=(ks == 1),
                                skip_group_check=True,
                            )
                        ot = opool.tile([128, TCH], BF16, tag="o")
                        nc.vector.tensor_copy(ot[:], ps[:])
                        last = tcix == n_chunks - 1
                        eng = nc.scalar if last else nc.gpsimd
                        eng.dma_start(
                            y[ts0 + tt * 128 : ts0 + (tt + 1) * 128,
                              nn * TCH : (nn + 1) * TCH],
                            ot[:],
                        )
                    units.append(u)
            return units

        # ---- main loop: attention per chunk with filler interleaving ----
        for tcix in range(n_chunks):
            ts0 = tcix * TCH
            fillers = []
            if tcix + 1 < n_chunks:
                fillers += qkv_units(tcix + 1, load_xt(tcix + 1))
            if tcix >= 1:
                fillers += fc_units(tcix - 1)

            kimax = tcix * 4 + 3
            npairs = 2 * tcix + 2
            slots = 2 * npairs * 2
            state = {"s": 0, "f": 0}

            def tick():
                state["s"] += 1
                want = state["s"] * len(fillers) // slots
                while state["f"] < want:
                    fillers[state["f"]]()
                    state["f"] += 1

            attn_t = attnpool.tile([128, 2, TCH], BF16, tag="attn")
            attn_tiles[tcix] = attn_t
            for hp in range(2):
                heads = (2 * hp, 2 * hp + 1)
                ps_av = {}
                for h in heads:
                    ps_av[h] = avps.tile([128, TCH], F32, tag="av", name=f"av{h}")
                wts = {}
                pend = []

                tail = tcix == n_chunks - 1 and hp == 1
                mv = DH + 32 if tail else DH + 1

                def do_av(item):
                    h, pj = item
                    wt = wts[(h, pj)]
                    for u in range(2):
                        ki = 2 * pj + u
                        sx = max(0, (ki - 4 * tcix) * 128)
                        nc.tensor.matmul(
                            ps_av[h][0:mv, sx:TCH],
                            v_sb[:, ki, h, 0:mv],
                            wt[:, u, sx:TCH],
                            start=(ki == 0),
                            stop=(ki == kimax),
                            skip_group_check=True,
                        )

                def emit_half(h, c0):
                    pb = (h % 2) * 64
                    ho = h // 2
                    t1 = rpool.tile([32, 256], F32, tag="t1")
                    nc.vector.transpose(t1[:], ps_av[h][DH : DH + 32, c0 : c0 + 256])
                    t1v = t1.rearrange("p (j k) -> p j k", k=32)
                    rcp32 = rpool.tile([32, 8], F32, tag="rcp32")
                    nc.vector.reciprocal(rcp32[:], t1v[:, :, 0:1])
                    in2 = rpool.tile([32, 256], F32, tag="in2")
                    nc.vector.tensor_copy(
                        in2.rearrange("p (j k) -> p j k", k=32),
                        rcp32[:, :, None].to_broadcast((32, 8, 32)),
                    )
                    out2 = rpool.tile([32, 256], F32, tag="out2")
                    nc.vector.transpose(out2[:], in2[:])
                    nc.vector.tensor_mul(
                        attn_t[pb : pb + 32, ho, c0 : c0 + 256],
                        ps_av[h][0:32, c0 : c0 + 256],
                        out2[:],
                    )
                    nc.vector.tensor_mul(
                        attn_t[pb + 32 : pb + 64, ho, c0 : c0 + 256],
                        ps_av[h][32:DH, c0 : c0 + 256],
                        out2[:],
                    )

                def finish(item):
                    do_av(item)
                    if tail:
                        hh, pjj = item
                        if pjj == npairs - 2:
                            emit_half(hh, 0)
                        elif pjj == npairs - 1:
                            emit_half(hh, 256)

                for pj in range(npairs):
                    for h in heads:
                        pb = (h % 2) * 64
                        ho = h // 2
                        st = sps.tile([128, 2, TCH], F32, tag="s")
                        for u in range(2):
                            ki = 2 * pj + u
                            # columns left of the diagonal block are fully
                            # masked and never read downstream: skip them
                            m = ki - 4 * tcix
                            sx = 128 * m if m >= 1 else 0
                            nc.tensor.matmul(
                                st[:, u, sx:],
                                kT_sb[pb : pb + 64, ho, ki * 128 : (ki + 1) * 128],
                                qT_sb[pb : pb + 64, ho, ts0 + sx : ts0 + TCH],
                                start=True,
                                stop=True,
                                skip_group_check=True,
                            )
                        wt = wtpool.tile([128, 2, TCH], BF16, tag="wt")
                        m0 = 2 * pj - 4 * tcix  # block offset of the pair's first ki
                        if m0 >= 2:
                            # mostly-masked final pair: exp only the live columns
                            for u in range(2):
                                sx = (m0 + u) * 128
                                nc.scalar.activation(
                                    wt[:, u, sx:TCH],
                                    st[:, u, sx:TCH],
                                    mybir.ActivationFunctionType.Exp,
                                    scale=0.125,
                                )
                        else:
                            nc.scalar.activation(
                                wt[:],
                                st[:],
                                mybir.ActivationFunctionType.Exp,
                                scale=0.125,
                            )
                        for u in range(2):
                            ki = 2 * pj + u
                            m = ki - 4 * tcix
                            if m >= 0:
                                sx = m * 128
                                nc.vector.tensor_mul(
                                    wt[:, u, sx : sx + 128],
                                    wt[:, u, sx : sx + 128],
                                    mask_sb[:],
                                )
                        wts[(h, pj)] = wt
                        pend.append((h, pj))
                        if len(pend) > 4:
                            finish(pend.pop(0))
                        tick()
                while pend:
                    finish(pend.pop(0))

                for h in heads:
                    pb = (h % 2) * 64
                    ho = h // 2
                    if tail:
                        continue  # per-column-half chains emitted in the drain
                    # stage the AV psum to SBUF (frees the bank for the next
                    # head pair); transposed reciprocal via a DRAM bounce on
                    # the otherwise-idle scalar HW DGE queue
                    stage = rpool.tile([DH + 1, TCH], F32, tag="stage")
                    nc.vector.tensor_copy(stage[:], ps_av[h][0 : DH + 1, :])
                    d1 = dpool.tile([1, TCH], F32)
                    nc.sync.dma_start(d1[:], stage[DH : DH + 1, :])
                    rp = rpool.tile([128, 4], F32, tag="rp")
                    nc.sync.dma_start(
                        rp[:],
                        bass.AP(tensor=d1.tensor, offset=d1.offset, ap=[[4, 128], [1, 4]]),
                    )
                    rcp = rpool.tile([128, 4], F32, tag="rcp")
                    nc.vector.reciprocal(rcp[:], rp[:])
                    d2 = dpool.tile([1, TCH], F32)
                    nc.sync.dma_start(
                        bass.AP(tensor=d2.tensor, offset=d2.offset, ap=[[4, 128], [1, 4]]),
                        rcp[:],
                    )
                    rep = rpool.tile([64, TCH], F32, tag="rep")
                    nc.sync.dma_start(
                        rep[:],
                        bass.AP(tensor=d2.tensor, offset=d2.offset, ap=[[0, 64], [1, TCH]]),
                    )
                    nc.vector.tensor_mul(
                        attn_t[pb : pb + 64, ho, :], stage[0:DH, :], rep[:]
                    )

            while state["f"] < len(fillers):
                fillers[state["f"]]()
                state["f"] += 1

        for u in fc_units(n_chunks - 1):
            u()

    nc.compile()
    return nc


def _prep_core_inputs(x, w_qkv, b_qkv, w_fc, b_fc, core):
    b, g = core // 4, core % 4
    rq = slice(256 * g, 256 * g + 256)
    rk = slice(1024 + 256 * g, 1024 + 256 * g + 256)
    rv = slice(2048 + 256 * g, 2048 + 256 * g + 256)
    wcat = np.concatenate([w_qkv[rq], w_qkv[rk], w_qkv[rv]], axis=0)  # [768, 1024]
    bq, bk, bv = b_qkv[rq], b_qkv[rk], b_qkv[rv]
    import ml_dtypes

    bf16 = ml_dtypes.bfloat16
    # SBUF-tile layouts so every DMA moves full-line-rate contiguous runs:
    xT = x[b].T  # [C, T]
    xtl = xT.reshape(8, 128, T // TCH, TCH).transpose(2, 1, 0, 3)  # [tc, ci, co, t]
    wql = wcat.T.reshape(8, 128, 768).transpose(1, 0, 2)  # [ci, co, o]
    wfcl = w_fc[:, 256 * g : 256 * g + 256].T.reshape(2, 128, 1024).transpose(1, 0, 2)
    return {
        "xtl": np.ascontiguousarray(xtl).astype(bf16),
        "wql": np.ascontiguousarray(wql).astype(bf16),
        "bqk": np.ascontiguousarray(
            np.stack([bq[0:128], bq[128:256], bk[0:128], bk[128:256]], axis=1)
        ),
        "bv_rep": np.ascontiguousarray(np.broadcast_to(bv, (128, 256))),
        "wfcl": np.ascontiguousarray(wfcl).astype(bf16),
        "mask": np.triu(np.ones((128, 128), dtype=np.float32)).astype(bf16),
    }


def kernel(x, w_qkv, b_qkv, w_fc, b_fc):
    global LAST_RESULT
    x = np.asarray(x, dtype=np.float32)
    w_qkv = np.asarray(w_qkv, dtype=np.float32)
    b_qkv = np.asarray(b_qkv, dtype=np.float32)
    w_fc = np.asarray(w_fc, dtype=np.float32)
    b_fc = np.asarray(b_fc, dtype=np.float32)

    if "nc" not in _BUILD_CACHE:
        _BUILD_CACHE["nc"] = build()
    nc = _BUILD_CACHE["nc"]

    in_maps = [
        _prep_core_inputs(x, w_qkv, b_qkv, w_fc, b_fc, core) for core in range(8)
    ]
    res = run_bass_kernel_spmd(
        nc,
        in_maps,
        core_ids=list(range(8)),
        trace=bool(os.environ.get("MHA_TRACE")),
    )
    LAST_RESULT = res

    out = np.empty((B, T, C), dtype=np.float32)
    for b in range(B):
        acc = res.results[4 * b]["y"].astype(np.float32)
        for g in range(1, 4):
            acc = acc + res.results[4 * b + g]["y"].astype(np.float32)
        out[b] = acc + b_fc
    return out
